# revision 17
# baseline (speedup 1.0000x reference)
"""CPI-MPNN (molecule MPNN + protein CNN + FC head) Trainium2 kernel.

Self-contained: hardcodes all shapes. Shards the batch (128) across 8
NeuronCores (16 samples each), replicates the small weights.

Strategy (v2):
  - All-bf16 datapath (PE full rate at any N, unlike fp32r which needs
    N>=256 — drops the 200->256 padding of v1).
  - conv0 partition-packed: taps 0+1 share one matmul via a host-built
    duplicate of the protein activation shifted by one position in the
    spare 50 partitions (K=50 -> 100), 3 matmuls -> 2 per chunk.
  - MPNN without PE transposes: the neighbor-sum is computed H-major by
    using the message as the stationary operand (N=96 per matmul), so
    the W_h contraction consumes it directly.
  - Helper-engine rebalance: conv activations on DVE (scalar ACTIVATE
    was 54% busy in v1), psum->sbuf copies split scalar/DVE, mean-pool
    scale folded into W_o (relu positive homogeneity).
"""

import numpy as np
from contextlib import ExitStack

import concourse.bass as bass
import concourse.tile as tile
from concourse import bacc, mybir
from concourse.bass_utils import run_bass_kernel_spmd

F32 = mybir.dt.float32
BF16 = mybir.dt.bfloat16
AF = mybir.ActivationFunctionType
ALU = mybir.AluOpType

# model dims
H = 200
ATOM_FDIM = 39
BOND_FDIM = 11
B, NA, NB = 128, 48, 96
L, VOCAB = 1000, 26

NCORES = 8
M = B // NCORES          # samples (molecules+proteins) per core (16)
SEG = 1006               # 3 + 1000 + 3 padded segment
PAD = 3
NCH = 500                # conv free-dim chunk (2 per sample)

_CACHE = {}


def _build_nc():
    nc = bacc.Bacc("TRN2", target_bir_lowering=False, debug=False)

    # ---- DRAM inputs (per core) ----
    # protein activations: [100, SEG] bf16 per sample; rows 0:50 are the
    # embedded sequence (conv pads baked in), rows 50:100 the same
    # shifted one position left, so conv0 taps 0+1 contract in a single
    # K=100 matmul.
    d_x0 = [nc.dram_tensor(f"x0d{g}", [100, SEG], BF16, kind="ExternalInput")
            for g in range(M)]
    d_fbt = nc.dram_tensor("fbt", [50, M, 96], BF16, kind="ExternalInput")
    d_cat1 = nc.dram_tensor("cat1", [40, M, 48], BF16, kind="ExternalInput")
    d_abt = nc.dram_tensor("abt", [96, M, 96], BF16, kind="ExternalInput")
    d_aat = nc.dram_tensor("aat", [96, M, 48], BF16, kind="ExternalInput")

    d_wi = nc.dram_tensor("wi", [50, 200], BF16, kind="ExternalInput")
    d_wha = nc.dram_tensor("wha", [128, 200], BF16, kind="ExternalInput")
    d_whb = nc.dram_tensor("whb", [72, 200], BF16, kind="ExternalInput")
    d_wo1 = nc.dram_tensor("wo1", [40, 200], BF16, kind="ExternalInput")
    d_wo2 = nc.dram_tensor("wo2", [128, 200], BF16, kind="ExternalInput")
    d_wo3 = nc.dram_tensor("wo3", [72, 200], BF16, kind="ExternalInput")
    d_w0p = nc.dram_tensor("w0p", [100, 96], BF16, kind="ExternalInput")
    d_w0c = nc.dram_tensor("w0c", [50, 96], BF16, kind="ExternalInput")
    d_b0 = nc.dram_tensor("b0", [96, 1], F32, kind="ExternalInput")
    d_w1 = nc.dram_tensor("w1", [96, 5, 128], BF16, kind="ExternalInput")
    d_b1 = nc.dram_tensor("b1", [128, 1], F32, kind="ExternalInput")
    d_w2a = nc.dram_tensor("w2a", [128, 7, 128], BF16, kind="ExternalInput")
    d_w2b = nc.dram_tensor("w2b", [128, 7, 72], BF16, kind="ExternalInput")
    d_b2a = nc.dram_tensor("b2a", [128, 1], F32, kind="ExternalInput")
    d_b2b = nc.dram_tensor("b2b", [72, 1], F32, kind="ExternalInput")
    d_fc0 = [nc.dram_tensor(f"fc0{k}", [dim, 200], BF16, kind="ExternalInput")
             for k, dim in (("a", 128), ("b", 72), ("c", 128), ("d", 72))]
    d_fc0ba = nc.dram_tensor("fc0ba", [128, 1], F32, kind="ExternalInput")
    d_fc0bb = nc.dram_tensor("fc0bb", [72, 1], F32, kind="ExternalInput")
    d_fc1a = nc.dram_tensor("fc1a", [128, 100], BF16, kind="ExternalInput")
    d_fc1b = nc.dram_tensor("fc1b", [72, 100], BF16, kind="ExternalInput")
    d_fc1bias = nc.dram_tensor("fc1bias", [100, 1], F32, kind="ExternalInput")
    d_fc2w = nc.dram_tensor("fc2w", [100, 1], BF16, kind="ExternalInput")
    d_fc2b = nc.dram_tensor("fc2b", [1, 1], F32, kind="ExternalInput")
    d_ones = nc.dram_tensor("ones48", [48, 1], BF16, kind="ExternalInput")

    d_out = nc.dram_tensor("out", [1, M], F32, kind="ExternalOutput")

    with tile.TileContext(nc) as tc, ExitStack() as ctx:
        cst = ctx.enter_context(tc.tile_pool(name="cst", bufs=1))
        sbs = ctx.enter_context(tc.tile_pool(name="sbs", bufs=1))
        tmp = ctx.enter_context(tc.tile_pool(name="tmp", bufs=1))
        xp = ctx.enter_context(tc.tile_pool(name="xp", bufs=1))
        pp = ctx.enter_context(tc.tile_pool(name="pp", bufs=1, space="PSUM"))

        # ---- PE warm-up ----
        # The HAM clock-gate needs ~3us of continuous PE activity to
        # reach full rate, and the first input DMAs land ~11us in. Run
        # throwaway matmuls on a zeroed tile during the DMA wait so the
        # real stream starts at full clock.
        warm = cst.tile([128, 628], BF16, tag="warm")
        nc.gpsimd.memset(warm[:], 0.0)
        warm_ps = pp.tile([128, NCH], F32, tag="cv", bufs=3, name="warm_ps")

        def emit_warm():
            nc.tensor.matmul(warm_ps[:], warm[:, 0:128], warm[:, 128:628],
                             start=True, stop=True)

        for _ in range(3):
            emit_warm()

        # ---- constants ----
        def const_tile(dram, shape, dtype=BF16, name=None, eng=None):
            t = cst.tile(shape, dtype, tag=name or dram.name)
            (eng or nc.sync).dma_start(t[:], dram.ap())
            return t

        GM = 4
        fbt_g, abt_g, aat_g, cat1_g = {}, {}, {}, {}

        def fbt_dma(g):
            t = cst.tile([50, GM * 96], BF16, tag=f"fbt{g}")
            nc.sync.dma_start(t[:].rearrange("p (m i) -> p m i", i=96),
                              d_fbt.ap()[:, GM * g:GM * (g + 1), :])
            fbt_g[g] = t

        def abt_dma(g):
            t = cst.tile([96, GM * 96], BF16, tag=f"abt{g}")
            nc.sync.dma_start(t[:].rearrange("p (m i) -> p m i", i=96),
                              d_abt.ap()[:, GM * g:GM * (g + 1), :])
            abt_g[g] = t

        def aat_cat_dma(g, eng):
            t = cst.tile([96, GM * 48], BF16, tag=f"aat{g}")
            eng.dma_start(t[:].rearrange("p (m i) -> p m i", i=48),
                          d_aat.ap()[:, GM * g:GM * (g + 1), :])
            aat_g[g] = t
            t = cst.tile([40, GM * 48], BF16, tag=f"cat1{g}")
            eng.dma_start(t[:].rearrange("p (m i) -> p m i", i=48),
                          d_cat1.ap()[:, GM * g:GM * (g + 1), :])
            cat1_g[g] = t

        # ACT queue: only the first protein buffer up front — every other
        # startup DMA goes on the SP queue so the scalar engine is free
        # for the first MPNN relus (DMA descriptor issue costs ~0.8us of
        # engine time each).
        x0_bufs = {}

        def x0_dma(s):
            if s in x0_bufs or s >= M:
                return
            t = xp.tile([100, SEG], BF16, tag=f"x0s{s}")
            eng = nc.scalar if s == 0 else nc.sync
            eng.dma_start(t[:], d_x0[s].ap())
            x0_bufs[s] = t

        x0_dma(0)

        # SP queue: mol-group-0 inputs first (binput/iter critical path),
        # then W_h / conv2 weights, then later groups + FC weights.
        wi_t = const_tile(d_wi, [50, 200])
        fbt_dma(0)
        abt_dma(0)
        w0p_t = const_tile(d_w0p, [100, 96], eng=nc.sync)
        w0c_t = const_tile(d_w0c, [50, 96], eng=nc.sync)
        b0_t = const_tile(d_b0, [96, 1], F32, eng=nc.sync)
        x0_dma(1)
        w1_t = cst.tile([96, 5 * 128], BF16, tag="w1")
        nc.sync.dma_start(w1_t[:].rearrange("p (t o) -> p t o", o=128),
                          d_w1.ap())
        b1_t = const_tile(d_b1, [128, 1], F32, eng=nc.sync)
        wha_t = const_tile(d_wha, [128, 200])
        whb_t = const_tile(d_whb, [72, 200])
        w2a_t = cst.tile([128, 7 * 128], BF16, tag="w2a")
        nc.sync.dma_start(w2a_t[:].rearrange("p (t o) -> p t o", o=128),
                          d_w2a.ap())
        w2b_t = cst.tile([128, 7 * 72], BF16, tag="w2b")
        nc.sync.dma_start(w2b_t[:].rearrange("p (t o) -> p t o", o=72),
                          d_w2b.ap())
        aat_cat_dma(0, nc.sync)
        wo1_t = const_tile(d_wo1, [40, 200], eng=nc.sync)
        wo2_t = const_tile(d_wo2, [128, 200], eng=nc.sync)
        wo3_t = const_tile(d_wo3, [72, 200], eng=nc.sync)
        ones_t = const_tile(d_ones, [48, 1], eng=nc.sync)
        b2a_t = const_tile(d_b2a, [128, 1], F32, eng=nc.sync)
        b2b_t = const_tile(d_b2b, [72, 1], F32, eng=nc.sync)
        fbt_dma(1)
        abt_dma(1)
        aat_cat_dma(1, nc.sync)
        fbt_dma(2)
        abt_dma(2)
        aat_cat_dma(2, nc.sync)
        fbt_dma(3)
        abt_dma(3)
        aat_cat_dma(3, nc.sync)
        fc0_t = [const_tile(d, [dim, 200], eng=nc.sync) for d, dim in
                 zip(d_fc0, (128, 72, 128, 72))]
        fc0ba_t = const_tile(d_fc0ba, [128, 1], F32, eng=nc.sync)
        fc0bb_t = const_tile(d_fc0bb, [72, 1], F32, eng=nc.sync)
        fc1a_t = const_tile(d_fc1a, [128, 100], eng=nc.sync)
        fc1b_t = const_tile(d_fc1b, [72, 100], eng=nc.sync)
        fc1bias_t = const_tile(d_fc1bias, [100, 1], F32, eng=nc.sync)
        fc2w_t = const_tile(d_fc2w, [100, 1], eng=nc.sync)
        fc2b_t = const_tile(d_fc2b, [1, 1], F32, eng=nc.sync)


        # static outputs of the two towers, feature-major [feat, M]
        embT1 = sbs.tile([128, M], BF16, tag="embT1")
        embT2 = sbs.tile([72, M], BF16, tag="embT2")
        embT1f = sbs.tile([128, M], F32, tag="embT1f")
        embT2f = sbs.tile([72, M], F32, tag="embT2f")
        prT1p = sbs.tile([128, M], F32, tag="prT1p")
        prT2p = sbs.tile([72, M], F32, tag="prT2p")

        # ================= per-molecule MPNN (staged) =================
        # Every stage is a generator that yields after each PE matmul so
        # the scheduler can weave conv (N=500) matmuls between the small
        # MPNN matmuls: the small matmuls' LDWEIGHTS then hide under the
        # conv matmuls' execution and the PE array duty cycle stays above
        # the HAM throttle threshold.
        mol_state = {}

        def gen_binput(m):
            g, r = m // GM, m % GM
            fb_m = fbt_g[g][:, r * 96:(r + 1) * 96]
            ps = pp.tile([96, 200], F32, tag="mp", bufs=3)
            nc.tensor.matmul(ps[:], fb_m, wi_t[:], start=True, stop=True)
            binp = sbs.tile([96, 200], F32, tag=f"binp{m}")
            nc.vector.tensor_copy(binp[:], ps[:])
            msg = sbs.tile([96, 200], BF16, tag=f"msg{m}")
            nc.scalar.activation(msg[:], ps[:], AF.Relu)
            mol_state[m] = (binp, msg)
            yield

        def gen_iter_pre(m):
            g, r = m // GM, m % GM
            ab_m = abt_g[g][:, r * 96:(r + 1) * 96]
            binp, msg = mol_state[m]
            # pa/pb share one PSUM bank slot; both are single-matmul
            # accumulation groups so the bank-granular pending-zero mark
            # of the second can't corrupt the first mid-group.
            nt = pp.tile([128, 192], F32, tag="nt", bufs=2)
            nc.tensor.matmul(nt[0:128, 0:96], msg[:, 0:128], ab_m,
                             start=True, stop=True)
            yield
            nc.tensor.matmul(nt[0:72, 96:192], msg[:, 128:200], ab_m,
                             start=True, stop=True)
            nTa = tmp.tile([128, 96], BF16, tag="nTa", bufs=6)
            nc.vector.tensor_copy(nTa[:], nt[0:128, 0:96])
            nTb = tmp.tile([72, 96], BF16, tag="nTb", bufs=6)
            nc.vector.tensor_copy(nTb[:], nt[0:72, 96:192])
            mol_state[m] = (binp, msg, nTa, nTb)
            yield

        def gen_iter_post(m):
            binp, msg, nTa, nTb = mol_state[m]
            ps = pp.tile([96, 200], F32, tag="mp", bufs=3)
            nc.tensor.matmul(ps[:], nTa[:], wha_t[:], start=True, stop=False)
            yield
            nc.tensor.matmul(ps[:], nTb[:], whb_t[:], start=False, stop=True)
            tm = tmp.tile([96, 200], F32, tag="mtmp", bufs=3)
            nc.vector.tensor_add(tm[:], ps[:], binp[:])
            nc.scalar.activation(msg[:], tm[:], AF.Relu)
            mol_state[m] = (binp, msg)
            yield

        def gen_atom(m):
            g, r = m // GM, m % GM
            aa_m = aat_g[g][:, r * 48:(r + 1) * 48]
            c1_m = cat1_g[g][:, r * 48:(r + 1) * 48]
            binp, msg = mol_state[m]
            pT = pp.tile([128, 96], F32, tag="nt", bufs=2)
            nc.tensor.matmul(pT[0:128, 0:48], msg[:, 0:128], aa_m,
                             start=True, stop=True)
            yield
            nc.tensor.matmul(pT[0:72, 48:96], msg[:, 128:200], aa_m,
                             start=True, stop=True)
            nat1 = tmp.tile([128, 48], BF16, tag="nat1", bufs=3)
            nc.scalar.copy(nat1[:], pT[0:128, 0:48])
            nat2 = tmp.tile([72, 48], BF16, tag="nat2", bufs=3)
            nc.scalar.copy(nat2[:], pT[0:72, 48:96])
            yield
            # atom hidden state computed H-major (out [h, atom]) so the
            # atom mean collapses into the activation's accum_out; W_o
            # is pre-scaled by 1/48 on host so the mean is a plain sum.
            psa = pp.tile([128, 48], F32, tag="nt", bufs=2, name="psAHa")
            nc.tensor.matmul(psa[:], wo1_t[:, 0:128], c1_m,
                             start=True, stop=False)
            yield
            nc.tensor.matmul(psa[:], wo2_t[:, 0:128], nat1[:],
                             start=False, stop=False)
            yield
            nc.tensor.matmul(psa[:], wo3_t[:, 0:128], nat2[:],
                             start=False, stop=True)
            ra = tmp.tile([128, 48], BF16, tag="reluh", bufs=3, name="ra")
            nc.scalar.activation(ra[:], psa[:], AF.Relu,
                                 accum_out=embT1f[:, m:m + 1])
            yield
            psb = pp.tile([72, 48], F32, tag="nt", bufs=2, name="psAHb")
            nc.tensor.matmul(psb[:], wo1_t[:, 128:200], c1_m,
                             start=True, stop=False)
            yield
            nc.tensor.matmul(psb[:], wo2_t[:, 128:200], nat1[:],
                             start=False, stop=False)
            yield
            nc.tensor.matmul(psb[:], wo3_t[:, 128:200], nat2[:],
                             start=False, stop=True)
            rb = tmp.tile([72, 48], BF16, tag="reluh", bufs=3, name="rb")
            nc.scalar.activation(rb[:], psb[:], AF.Relu,
                                 accum_out=embT2f[:, m:m + 1])
            yield

        # ================= per-sample protein conv tower =================
        sample_state = {}

        def gen_conv0(s):
            x0 = x0_bufs[s]
            x1 = xp.tile([96, SEG], BF16, tag="x1", bufs=3)
            nc.gpsimd.memset(x1[:, 0:PAD], 0.0)
            nc.gpsimd.memset(x1[:, PAD + 1000:SEG], 0.0)
            for c in range(2):
                base = PAD + c * NCH
                ps = pp.tile([96, NCH], F32, tag="cv", bufs=3)
                nc.tensor.matmul(ps[:], w0p_t[:],
                                 x0[:, base - 1:base - 1 + NCH],
                                 start=True, stop=False)
                yield
                nc.tensor.matmul(ps[:], w0c_t[:],
                                 x0[0:50, base + 1:base + 1 + NCH],
                                 start=False, stop=True)
                nc.scalar.activation(x1[:, base:base + NCH], ps[:],
                                     AF.Relu, bias=b0_t[:])
                yield
            x0_dma(s + 2)
            sample_state[s] = [x1, None, None, None]

        def gen_conv1(s, c):
            st = sample_state[s]
            x1 = st[0]
            if c == 0:
                x2 = xp.tile([128, SEG], BF16, tag="x2", bufs=4)
                nc.gpsimd.memset(x2[:, 0:PAD], 0.0)
                nc.gpsimd.memset(x2[:, PAD + 1000:SEG], 0.0)
                st[1] = x2
            x2 = st[1]
            base = PAD + c * NCH
            ps = pp.tile([128, NCH], F32, tag="cv", bufs=3)
            for t in range(5):
                nc.tensor.matmul(
                    ps[:], w1_t[:, t * 128:(t + 1) * 128],
                    x1[:, base + t - 2:base + t - 2 + NCH],
                    start=(t == 0), stop=(t == 4))
                if t < 4:
                    yield
            nc.scalar.activation(x2[:, base:base + NCH], ps[:],
                                 AF.Relu, bias=b1_t[:])
            yield

        def gen_conv2(s, c, half):
            st = sample_state[s]
            x2 = st[1]
            if half == 0:
                if c == 0:
                    st[2] = tmp.tile([128, 2], F32, tag="mxA", bufs=3,
                                     name="mxA")
                    st[3] = tmp.tile([72, 2], F32, tag="mxB", bufs=3,
                                     name="mxB")
                w_t, wid, mx = w2a_t, 128, st[2]
            else:
                w_t, wid, mx = w2b_t, 72, st[3]
            base = PAD + c * NCH
            ps = pp.tile([wid, NCH], F32, tag="cv", bufs=3)
            for t in range(7):
                nc.tensor.matmul(
                    ps[:], w_t[:, t * wid:(t + 1) * wid],
                    x2[:, base + t - 3:base + t - 3 + NCH],
                    start=(t == 0), stop=(t == 6))
                if t < 6:
                    yield
            nc.vector.reduce_max(mx[:, c:c + 1], ps[:],
                                 axis=mybir.AxisListType.X)
            if c == 1:
                dst = prT1p if half == 0 else prT2p
                nc.vector.reduce_max(dst[:, s:s + 1], mx[:],
                                     axis=mybir.AxisListType.X)
                if half == 1:
                    sample_state.pop(s)
            yield

        # FC head first-layer matmuls on the molecule embeddings can run
        # before the last conv pieces finish; woven in as mol pieces.
        # PSUM tiles allocated lazily (inside the generator) so the nt
        # slot rotation used by iter_pre/atom is not disturbed.
        fc_ps = {}

        def gen_fc_early():
            nc.gpsimd.tensor_copy(embT1[:], embT1f[:])
            nc.gpsimd.tensor_copy(embT2[:], embT2f[:])
            fc_ps["a"] = pp.tile([128, M], F32, tag="nt", bufs=2, name="ps0a")
            fc_ps["b"] = pp.tile([72, M], F32, tag="nt", bufs=2, name="ps0b")
            rhs2 = (embT1, embT2)
            for k in range(2):
                nc.tensor.matmul(fc_ps["a"][:], fc0_t[k][:, 0:128],
                                 rhs2[k][:], start=(k == 0), stop=False)
                yield
            for k in range(2):
                nc.tensor.matmul(fc_ps["b"][:], fc0_t[k][:, 128:200],
                                 rhs2[k][:], start=(k == 0), stop=False)
                yield

        # ---- build the two global piece streams and weave them ----
        conv_pieces = []
        mol_pieces = []
        for g in range(4):
            s0, s1, s2, s3 = (4 * g + i for i in range(4))
            mols = [GM * g + r for r in range(GM)]
            for stage in (gen_binput, gen_iter_pre, gen_iter_post,
                          gen_iter_pre, gen_iter_post, gen_atom):
                for m in mols:
                    mol_pieces.append(stage(m))
            conv_pieces += [
                gen_conv0(s0), gen_conv0(s1),
                gen_conv1(s0, 0), gen_conv1(s0, 1),
                gen_conv1(s1, 0), gen_conv1(s1, 1),
                gen_conv2(s0, 0, 0), gen_conv2(s0, 0, 1),
                gen_conv2(s0, 1, 0), gen_conv2(s0, 1, 1),
                gen_conv0(s2), gen_conv1(s2, 0), gen_conv1(s2, 1),
                gen_conv2(s1, 0, 0), gen_conv2(s1, 0, 1),
                gen_conv2(s1, 1, 0), gen_conv2(s1, 1, 1),
                gen_conv0(s3), gen_conv1(s3, 0), gen_conv1(s3, 1),
                gen_conv2(s2, 0, 0), gen_conv2(s2, 0, 1),
                gen_conv2(s2, 1, 0), gen_conv2(s2, 1, 1),
                gen_conv2(s3, 0, 0), gen_conv2(s3, 0, 1),
                gen_conv2(s3, 1, 0), gen_conv2(s3, 1, 1),
            ]
        mol_pieces.append(gen_fc_early())

        def stream(pieces):
            for p in pieces:
                yield from p

        cs = stream(conv_pieces)
        ms = stream(mol_pieces)
        # lead with the first mol group's binput+iter_pre (their inputs
        # land before any conv input does); warmup matmuls fill the
        # dependency stalls of this phase
        for _ in range(12):
            next(ms, None)
            emit_warm()
        conv_alive = mol_alive = True
        credit = 0.0
        RATIO = 672.0 / 276.0
        while conv_alive or mol_alive:
            credit += RATIO
            while credit >= 1.0 and conv_alive:
                credit -= 1.0
                if next(cs, StopIteration) is StopIteration:
                    conv_alive = False
            if mol_alive and next(ms, StopIteration) is StopIteration:
                mol_alive = False
            if not conv_alive:
                credit = 0.0

        # maxpool -> bias -> relu (monotone, so pool-first is exact)
        prT1 = sbs.tile([128, M], BF16, tag="prT1")
        nc.vector.tensor_scalar(prT1[:], prT1p[:], b2a_t[:], 0.0,
                                op0=ALU.add, op1=ALU.max)
        prT2 = sbs.tile([72, M], BF16, tag="prT2")
        nc.vector.tensor_scalar(prT2[:], prT2p[:], b2b_t[:], 0.0,
                                op0=ALU.add, op1=ALU.max)

        # ================= FC head (tail: protein contributions) =======
        rhs4 = (embT1, embT2, prT1, prT2)
        for k in (2, 3):
            nc.tensor.matmul(fc_ps["a"][:], fc0_t[k][:, 0:128], rhs4[k][:],
                             start=False, stop=(k == 3))
        h0a = tmp.tile([128, M], BF16, tag="h0a")
        nc.scalar.activation(h0a[:], fc_ps["a"][:], AF.Relu, bias=fc0ba_t[:])
        for k in (2, 3):
            nc.tensor.matmul(fc_ps["b"][:], fc0_t[k][:, 128:200], rhs4[k][:],
                             start=False, stop=(k == 3))
        h0b = tmp.tile([72, M], BF16, tag="h0b")
        nc.scalar.activation(h0b[:], fc_ps["b"][:], AF.Relu, bias=fc0bb_t[:])

        ps1 = pp.tile([100, M], F32, tag="nt", bufs=2)
        nc.tensor.matmul(ps1[:], fc1a_t[:], h0a[:], start=True, stop=False)
        nc.tensor.matmul(ps1[:], fc1b_t[:], h0b[:], start=False, stop=True)
        h1 = tmp.tile([100, M], BF16, tag="h1")
        nc.scalar.activation(h1[:], ps1[:], AF.Relu, bias=fc1bias_t[:])

        ps2 = pp.tile([1, M], F32, tag="nt", bufs=2)
        nc.tensor.matmul(ps2[:], fc2w_t[:], h1[:], start=True, stop=True)
        outsb = tmp.tile([1, M], F32, tag="outsb")
        nc.scalar.add(outsb[:], ps2[:], fc2b_t[:, 0:1])
        nc.sync.dma_start(d_out.ap(), outsb[:])

    nc.compile()
    return nc


def _prep(inputs):
    """Host preprocessing: returns the 8 per-core in_maps."""
    import ml_dtypes
    f32 = np.float32
    bf16 = ml_dtypes.bfloat16
    fatoms = np.asarray(inputs["fatoms"], f32)
    fbonds = np.asarray(inputs["fbonds"], f32)
    agraph = np.asarray(inputs["agraph"])
    bgraph = np.asarray(inputs["bgraph"])
    pseq = np.asarray(inputs["protein_seq"])
    W_i = np.asarray(inputs["W_i"], f32)
    W_h = np.asarray(inputs["W_h"], f32)
    W_o_w = np.asarray(inputs["W_o_w"], f32)
    W_o_b = np.asarray(inputs["W_o_b"], f32)
    embp = np.asarray(inputs["embed_protein"], f32)

    # protein embeddings, channel-major, with the shifted duplicate for
    # the packed conv0 (rows 50:100 = rows 0:50 shifted one col left)
    pvT = np.ascontiguousarray(embp[pseq].transpose(0, 2, 1))  # (B, 50, L)
    x0d = np.zeros((B, 100, SEG), bf16)
    x0d[:, 0:50, PAD:PAD + L] = pvT.astype(bf16)
    x0d[:, 50:100, PAD - 1:PAD - 1 + L] = x0d[:, 0:50, PAD:PAD + L]

    # adjacency one-hots (counts; contraction-dim-major)
    ar = np.arange(B)[:, None, None]
    cntB = np.zeros((B, NB, NB), f32)
    np.add.at(cntB, (ar, np.arange(NB)[None, :, None], bgraph), 1.0)
    abt = np.ascontiguousarray(cntB.transpose(0, 2, 1))        # (B, j, i)
    cntA = np.zeros((B, NA, NB), f32)
    np.add.at(cntA, (ar, np.arange(NA)[None, :, None], agraph), 1.0)
    aat = np.ascontiguousarray(cntA.transpose(0, 2, 1))        # (B, j, a)

    fbT = fbonds.transpose(0, 2, 1)                            # (B, 50, 96)
    faT = fatoms.transpose(0, 2, 1)                            # (B, 39, 48)
    cat1 = np.concatenate([faT, np.ones((B, 1, NA), f32)], axis=1)  # (B,40,48)

    # W_o scaled by 1/48: the atom mean becomes a plain sum (relu is
    # positively homogeneous)
    wo1 = np.zeros((40, 200), f32)
    wo1[:39] = W_o_w[0:39] / 48.0
    wo1[39] = W_o_b / 48.0
    wo2 = W_o_w[39:167] / 48.0
    wo3 = W_o_w[167:239] / 48.0

    conv_w = [np.asarray(inputs[f"conv{i}_w"], f32) for i in range(3)]
    conv_b = [np.asarray(inputs[f"conv{i}_b"], f32) for i in range(3)]
    w0p = np.concatenate([conv_w[0][:, :, 0].T, conv_w[0][:, :, 1].T], axis=0)
    w0c = np.ascontiguousarray(conv_w[0][:, :, 2].T)           # (50, 96)
    w1 = np.ascontiguousarray(conv_w[1].transpose(1, 2, 0))    # (96, 5, 128)
    w2 = np.ascontiguousarray(conv_w[2].transpose(1, 2, 0))    # (128, 7, 200)

    fcw = [np.asarray(inputs[f"fc{i}_w"], f32) for i in range(3)]
    fcb = [np.asarray(inputs[f"fc{i}_b"], f32) for i in range(3)]

    shared_bf = {
        "wi": W_i, "wha": W_h[0:128], "whb": W_h[128:200],
        "wo1": wo1, "wo2": wo2, "wo3": wo3,
        "w0p": w0p, "w0c": w0c, "w1": w1,
        "w2a": w2[:, :, 0:128], "w2b": w2[:, :, 128:200],
        "fc0a": fcw[0][0:128], "fc0b": fcw[0][128:200],
        "fc0c": fcw[0][200:328], "fc0d": fcw[0][328:400],
        "fc1a": fcw[1][0:128], "fc1b": fcw[1][128:200],
        "fc2w": fcw[2], "ones48": np.ones((48, 1), f32),
    }
    shared = {k: np.ascontiguousarray(v).astype(bf16)
              for k, v in shared_bf.items()}
    shared.update({
        "b0": conv_b[0].reshape(96, 1),
        "b1": conv_b[1].reshape(128, 1),
        "b2a": conv_b[2][0:128].reshape(128, 1),
        "b2b": conv_b[2][128:200].reshape(72, 1),
        "fc0ba": fcb[0][0:128].reshape(128, 1),
        "fc0bb": fcb[0][128:200].reshape(72, 1),
        "fc1bias": fcb[1].reshape(100, 1),
        "fc2b": fcb[2].reshape(1, 1),
    })
    for k in ("b0", "b1", "b2a", "b2b", "fc0ba", "fc0bb", "fc1bias", "fc2b"):
        shared[k] = np.ascontiguousarray(shared[k], f32)

    in_maps = []
    for c in range(NCORES):
        lo = c * M
        im = dict(shared)
        for g in range(M):
            im[f"x0d{g}"] = np.ascontiguousarray(x0d[lo + g])
        im["fbt"] = np.ascontiguousarray(
            fbT[lo:lo + M].transpose(1, 0, 2)).astype(bf16)
        im["cat1"] = np.ascontiguousarray(
            cat1[lo:lo + M].transpose(1, 0, 2)).astype(bf16)
        im["abt"] = np.ascontiguousarray(
            abt[lo:lo + M].transpose(1, 0, 2)).astype(bf16)
        im["aat"] = np.ascontiguousarray(
            aat[lo:lo + M].transpose(1, 0, 2)).astype(bf16)
        in_maps.append(im)
    return in_maps


def get_nc():
    if "nc" not in _CACHE:
        _CACHE["nc"] = _build_nc()
    return _CACHE["nc"]


def kernel(**inputs) -> np.ndarray:
    nc = get_nc()
    in_maps = _prep(inputs)
    res = run_bass_kernel_spmd(nc, in_maps, core_ids=list(range(NCORES)))
    outs = [res.results[c]["out"].reshape(M, 1) for c in range(NCORES)]
    return np.concatenate(outs, axis=0).astype(np.float32)


# revision 18
# speedup vs baseline: 1.0384x; 1.0384x over previous
"""CPI-MPNN (molecule MPNN + protein CNN + FC head) Trainium2 kernel.

Self-contained: hardcodes all shapes. Shards the batch (128) across 8
NeuronCores (16 samples each), replicates the small weights.

Strategy (v2):
  - All-bf16 datapath (PE full rate at any N, unlike fp32r which needs
    N>=256 — drops the 200->256 padding of v1).
  - conv0 partition-packed: taps 0+1 share one matmul via a host-built
    duplicate of the protein activation shifted by one position in the
    spare 50 partitions (K=50 -> 100), 3 matmuls -> 2 per chunk.
  - MPNN without PE transposes: the neighbor-sum is computed H-major by
    using the message as the stationary operand (N=96 per matmul), so
    the W_h contraction consumes it directly.
  - Helper-engine rebalance: conv activations on DVE (scalar ACTIVATE
    was 54% busy in v1), psum->sbuf copies split scalar/DVE, mean-pool
    scale folded into W_o (relu positive homogeneity).
"""

import numpy as np
from contextlib import ExitStack

import concourse.bass as bass
import concourse.tile as tile
from concourse import bacc, mybir
from concourse.bass_utils import run_bass_kernel_spmd

F32 = mybir.dt.float32
BF16 = mybir.dt.bfloat16
AF = mybir.ActivationFunctionType
ALU = mybir.AluOpType

# model dims
H = 200
ATOM_FDIM = 39
BOND_FDIM = 11
B, NA, NB = 128, 48, 96
L, VOCAB = 1000, 26

NCORES = 8
M = B // NCORES          # samples (molecules+proteins) per core (16)
SEG = 1006               # 3 + 1000 + 3 padded segment
PAD = 3
NCH = 500                # conv free-dim chunk (2 per sample)

_CACHE = {}


def _build_nc():
    nc = bacc.Bacc("TRN2", target_bir_lowering=False, debug=False)

    # ---- DRAM inputs (per core) ----
    # protein activations: [100, SEG] bf16 per sample; rows 0:50 are the
    # embedded sequence (conv pads baked in), rows 50:100 the same
    # shifted one position left, so conv0 taps 0+1 contract in a single
    # K=100 matmul.
    d_x0 = [nc.dram_tensor(f"x0d{g}", [100, SEG], BF16, kind="ExternalInput")
            for g in range(M)]
    d_fbt = nc.dram_tensor("fbt", [50, M, 96], BF16, kind="ExternalInput")
    d_cat1 = nc.dram_tensor("cat1", [40, M, 48], BF16, kind="ExternalInput")
    d_abt = nc.dram_tensor("abt", [96, M, 96], BF16, kind="ExternalInput")
    d_aat = nc.dram_tensor("aat", [96, M, 48], BF16, kind="ExternalInput")

    d_wi = nc.dram_tensor("wi", [50, 200], BF16, kind="ExternalInput")
    d_wha = nc.dram_tensor("wha", [128, 200], BF16, kind="ExternalInput")
    d_whb = nc.dram_tensor("whb", [72, 200], BF16, kind="ExternalInput")
    d_wo1 = nc.dram_tensor("wo1", [40, 200], BF16, kind="ExternalInput")
    d_wo2 = nc.dram_tensor("wo2", [128, 200], BF16, kind="ExternalInput")
    d_wo3 = nc.dram_tensor("wo3", [72, 200], BF16, kind="ExternalInput")
    d_w0p = nc.dram_tensor("w0p", [100, 96], BF16, kind="ExternalInput")
    d_w0c = nc.dram_tensor("w0c", [50, 96], BF16, kind="ExternalInput")
    d_b0 = nc.dram_tensor("b0", [96, 1], F32, kind="ExternalInput")
    d_w1 = nc.dram_tensor("w1", [96, 5, 128], BF16, kind="ExternalInput")
    d_b1 = nc.dram_tensor("b1", [128, 1], F32, kind="ExternalInput")
    d_w2a = nc.dram_tensor("w2a", [128, 7, 128], BF16, kind="ExternalInput")
    d_w2b = nc.dram_tensor("w2b", [128, 7, 72], BF16, kind="ExternalInput")
    d_b2a = nc.dram_tensor("b2a", [128, 1], F32, kind="ExternalInput")
    d_b2b = nc.dram_tensor("b2b", [72, 1], F32, kind="ExternalInput")
    d_fc0 = [nc.dram_tensor(f"fc0{k}", [dim, 200], BF16, kind="ExternalInput")
             for k, dim in (("a", 128), ("b", 72), ("c", 128), ("d", 72))]
    d_fc0ba = nc.dram_tensor("fc0ba", [128, 1], F32, kind="ExternalInput")
    d_fc0bb = nc.dram_tensor("fc0bb", [72, 1], F32, kind="ExternalInput")
    d_fc1a = nc.dram_tensor("fc1a", [128, 100], BF16, kind="ExternalInput")
    d_fc1b = nc.dram_tensor("fc1b", [72, 100], BF16, kind="ExternalInput")
    d_fc1bias = nc.dram_tensor("fc1bias", [100, 1], F32, kind="ExternalInput")
    d_fc2w = nc.dram_tensor("fc2w", [100, 1], BF16, kind="ExternalInput")
    d_fc2b = nc.dram_tensor("fc2b", [1, 1], F32, kind="ExternalInput")
    d_ones = nc.dram_tensor("ones48", [48, 1], BF16, kind="ExternalInput")

    d_out = nc.dram_tensor("out", [1, M], F32, kind="ExternalOutput")

    with tile.TileContext(nc) as tc, ExitStack() as ctx:
        cst = ctx.enter_context(tc.tile_pool(name="cst", bufs=1))
        sbs = ctx.enter_context(tc.tile_pool(name="sbs", bufs=1))
        tmp = ctx.enter_context(tc.tile_pool(name="tmp", bufs=1))
        xp = ctx.enter_context(tc.tile_pool(name="xp", bufs=1))
        pp = ctx.enter_context(tc.tile_pool(name="pp", bufs=1, space="PSUM"))

        # ---- PE warm-up ----
        # The HAM clock-gate needs ~3us of continuous PE activity to
        # reach full rate, and the first input DMAs land ~11us in. Run
        # throwaway matmuls on a zeroed tile during the DMA wait so the
        # real stream starts at full clock.
        warm = cst.tile([128, 628], BF16, tag="warm")
        nc.gpsimd.memset(warm[:], 0.0)
        warm_ps = pp.tile([128, NCH], F32, tag="cv", bufs=3, name="warm_ps")

        def emit_warm():
            nc.tensor.matmul(warm_ps[:], warm[:, 0:128], warm[:, 128:628],
                             start=True, stop=True)

        for _ in range(3):
            emit_warm()

        # ---- constants ----
        def const_tile(dram, shape, dtype=BF16, name=None, eng=None):
            t = cst.tile(shape, dtype, tag=name or dram.name)
            (eng or nc.sync).dma_start(t[:], dram.ap())
            return t

        GM = 4
        fbt_g, abt_g, aat_g, cat1_g = {}, {}, {}, {}

        def fbt_dma(g):
            t = cst.tile([50, GM * 96], BF16, tag=f"fbt{g}")
            nc.sync.dma_start(t[:].rearrange("p (m i) -> p m i", i=96),
                              d_fbt.ap()[:, GM * g:GM * (g + 1), :])
            fbt_g[g] = t

        def abt_dma(g):
            t = cst.tile([96, GM * 96], BF16, tag=f"abt{g}")
            nc.sync.dma_start(t[:].rearrange("p (m i) -> p m i", i=96),
                              d_abt.ap()[:, GM * g:GM * (g + 1), :])
            abt_g[g] = t

        def aat_cat_dma(g, eng):
            t = cst.tile([96, GM * 48], BF16, tag=f"aat{g}")
            eng.dma_start(t[:].rearrange("p (m i) -> p m i", i=48),
                          d_aat.ap()[:, GM * g:GM * (g + 1), :])
            aat_g[g] = t
            t = cst.tile([40, GM * 48], BF16, tag=f"cat1{g}")
            eng.dma_start(t[:].rearrange("p (m i) -> p m i", i=48),
                          d_cat1.ap()[:, GM * g:GM * (g + 1), :])
            cat1_g[g] = t

        # ACT queue: only the first protein buffer up front — every other
        # startup DMA goes on the SP queue so the scalar engine is free
        # for the first MPNN relus (DMA descriptor issue costs ~0.8us of
        # engine time each).
        x0_bufs = {}

        def x0_dma(s):
            if s in x0_bufs or s >= M:
                return
            t = xp.tile([100, SEG], BF16, tag=f"x0s{s}")
            eng = nc.scalar if s % 2 == 0 else nc.sync
            eng.dma_start(t[:], d_x0[s].ap())
            x0_bufs[s] = t

        x0_dma(0)

        # SP queue: mol-group-0 inputs first (binput/iter critical path),
        # then W_h / conv2 weights, then later groups + FC weights.
        wi_t = const_tile(d_wi, [50, 200])
        fbt_dma(0)
        abt_dma(0)
        w0p_t = const_tile(d_w0p, [100, 96], eng=nc.sync)
        w0c_t = const_tile(d_w0c, [50, 96], eng=nc.sync)
        b0_t = const_tile(d_b0, [96, 1], F32, eng=nc.sync)
        x0_dma(1)
        w1_t = cst.tile([96, 5 * 128], BF16, tag="w1")
        nc.sync.dma_start(w1_t[:].rearrange("p (t o) -> p t o", o=128),
                          d_w1.ap())
        b1_t = const_tile(d_b1, [128, 1], F32, eng=nc.sync)
        wha_t = const_tile(d_wha, [128, 200])
        whb_t = const_tile(d_whb, [72, 200])
        w2a_t = cst.tile([128, 7 * 128], BF16, tag="w2a")
        nc.sync.dma_start(w2a_t[:].rearrange("p (t o) -> p t o", o=128),
                          d_w2a.ap())
        w2b_t = cst.tile([128, 7 * 72], BF16, tag="w2b")
        nc.sync.dma_start(w2b_t[:].rearrange("p (t o) -> p t o", o=72),
                          d_w2b.ap())
        aat_cat_dma(0, nc.sync)
        wo1_t = const_tile(d_wo1, [40, 200], eng=nc.sync)
        wo2_t = const_tile(d_wo2, [128, 200], eng=nc.sync)
        wo3_t = const_tile(d_wo3, [72, 200], eng=nc.sync)
        ones_t = const_tile(d_ones, [48, 1], eng=nc.sync)
        b2a_t = const_tile(d_b2a, [128, 1], F32, eng=nc.sync)
        b2b_t = const_tile(d_b2b, [72, 1], F32, eng=nc.sync)
        fbt_dma(1)
        abt_dma(1)
        aat_cat_dma(1, nc.sync)
        fbt_dma(2)
        abt_dma(2)
        aat_cat_dma(2, nc.sync)
        fbt_dma(3)
        abt_dma(3)
        aat_cat_dma(3, nc.sync)
        fc0_t = [const_tile(d, [dim, 200], eng=nc.sync) for d, dim in
                 zip(d_fc0, (128, 72, 128, 72))]
        fc0ba_t = const_tile(d_fc0ba, [128, 1], F32, eng=nc.sync)
        fc0bb_t = const_tile(d_fc0bb, [72, 1], F32, eng=nc.sync)
        fc1a_t = const_tile(d_fc1a, [128, 100], eng=nc.sync)
        fc1b_t = const_tile(d_fc1b, [72, 100], eng=nc.sync)
        fc1bias_t = const_tile(d_fc1bias, [100, 1], F32, eng=nc.sync)
        fc2w_t = const_tile(d_fc2w, [100, 1], eng=nc.sync)
        fc2b_t = const_tile(d_fc2b, [1, 1], F32, eng=nc.sync)


        # static outputs of the two towers, feature-major [feat, M]
        embT1 = sbs.tile([128, M], BF16, tag="embT1")
        embT2 = sbs.tile([72, M], BF16, tag="embT2")
        embT1f = sbs.tile([128, M], F32, tag="embT1f")
        embT2f = sbs.tile([72, M], F32, tag="embT2f")
        prT1p = sbs.tile([128, M], F32, tag="prT1p")
        prT2p = sbs.tile([72, M], F32, tag="prT2p")

        # ================= per-molecule MPNN (staged) =================
        # Every stage is a generator that yields after each PE matmul so
        # the scheduler can weave conv (N=500) matmuls between the small
        # MPNN matmuls: the small matmuls' LDWEIGHTS then hide under the
        # conv matmuls' execution and the PE array duty cycle stays above
        # the HAM throttle threshold.
        mol_state = {}

        def gen_binput(m):
            g, r = m // GM, m % GM
            fb_m = fbt_g[g][:, r * 96:(r + 1) * 96]
            ps = pp.tile([96, 200], F32, tag="mp", bufs=3)
            nc.tensor.matmul(ps[:], fb_m, wi_t[:], start=True, stop=True)
            binp = sbs.tile([96, 200], F32, tag=f"binp{m}")
            nc.vector.tensor_copy(binp[:], ps[:])
            msg = sbs.tile([96, 200], BF16, tag=f"msg{m}")
            nc.scalar.activation(msg[:], ps[:], AF.Relu)
            mol_state[m] = (binp, msg)
            yield

        def gen_iter_pre(m):
            g, r = m // GM, m % GM
            ab_m = abt_g[g][:, r * 96:(r + 1) * 96]
            binp, msg = mol_state[m]
            # pa/pb share one PSUM bank slot; both are single-matmul
            # accumulation groups so the bank-granular pending-zero mark
            # of the second can't corrupt the first mid-group.
            nt = pp.tile([128, 192], F32, tag="nt", bufs=2)
            nc.tensor.matmul(nt[0:128, 0:96], msg[:, 0:128], ab_m,
                             start=True, stop=True)
            yield
            nc.tensor.matmul(nt[0:72, 96:192], msg[:, 128:200], ab_m,
                             start=True, stop=True)
            nTa = tmp.tile([128, 96], BF16, tag="nTa", bufs=6)
            nc.vector.tensor_copy(nTa[:], nt[0:128, 0:96])
            nTb = tmp.tile([72, 96], BF16, tag="nTb", bufs=6)
            nc.vector.tensor_copy(nTb[:], nt[0:72, 96:192])
            mol_state[m] = (binp, msg, nTa, nTb)
            yield

        def gen_iter_post(m):
            binp, msg, nTa, nTb = mol_state[m]
            ps = pp.tile([96, 200], F32, tag="mp", bufs=3)
            nc.tensor.matmul(ps[:], nTa[:], wha_t[:], start=True, stop=False)
            yield
            nc.tensor.matmul(ps[:], nTb[:], whb_t[:], start=False, stop=True)
            tm = tmp.tile([96, 200], F32, tag="mtmp", bufs=3)
            nc.vector.tensor_add(tm[:], ps[:], binp[:])
            nc.scalar.activation(msg[:], tm[:], AF.Relu)
            mol_state[m] = (binp, msg)
            yield

        def gen_atom(m):
            g, r = m // GM, m % GM
            aa_m = aat_g[g][:, r * 48:(r + 1) * 48]
            c1_m = cat1_g[g][:, r * 48:(r + 1) * 48]
            binp, msg = mol_state[m]
            pT = pp.tile([128, 96], F32, tag="nt", bufs=2)
            nc.tensor.matmul(pT[0:128, 0:48], msg[:, 0:128], aa_m,
                             start=True, stop=True)
            yield
            nc.tensor.matmul(pT[0:72, 48:96], msg[:, 128:200], aa_m,
                             start=True, stop=True)
            nat1 = tmp.tile([128, 48], BF16, tag="nat1", bufs=3)
            nc.scalar.copy(nat1[:], pT[0:128, 0:48])
            nat2 = tmp.tile([72, 48], BF16, tag="nat2", bufs=3)
            nc.scalar.copy(nat2[:], pT[0:72, 48:96])
            yield
            # atom hidden state computed H-major (out [h, atom]) so the
            # atom mean collapses into the activation's accum_out; W_o
            # is pre-scaled by 1/48 on host so the mean is a plain sum.
            psa = pp.tile([128, 48], F32, tag="nt", bufs=2, name="psAHa")
            nc.tensor.matmul(psa[:], wo1_t[:, 0:128], c1_m,
                             start=True, stop=False)
            yield
            nc.tensor.matmul(psa[:], wo2_t[:, 0:128], nat1[:],
                             start=False, stop=False)
            yield
            nc.tensor.matmul(psa[:], wo3_t[:, 0:128], nat2[:],
                             start=False, stop=True)
            ra = tmp.tile([128, 48], BF16, tag="reluh", bufs=3, name="ra")
            nc.scalar.activation(ra[:], psa[:], AF.Relu,
                                 accum_out=embT1f[:, m:m + 1])
            yield
            psb = pp.tile([72, 48], F32, tag="nt", bufs=2, name="psAHb")
            nc.tensor.matmul(psb[:], wo1_t[:, 128:200], c1_m,
                             start=True, stop=False)
            yield
            nc.tensor.matmul(psb[:], wo2_t[:, 128:200], nat1[:],
                             start=False, stop=False)
            yield
            nc.tensor.matmul(psb[:], wo3_t[:, 128:200], nat2[:],
                             start=False, stop=True)
            rb = tmp.tile([72, 48], BF16, tag="reluh", bufs=3, name="rb")
            nc.scalar.activation(rb[:], psb[:], AF.Relu,
                                 accum_out=embT2f[:, m:m + 1])
            yield

        # ================= per-sample protein conv tower =================
        sample_state = {}

        def gen_conv0(s):
            x0 = x0_bufs[s]
            x1 = xp.tile([96, SEG], BF16, tag="x1", bufs=3)
            nc.gpsimd.memset(x1[:, 0:PAD], 0.0)
            nc.gpsimd.memset(x1[:, PAD + 1000:SEG], 0.0)
            for c in range(2):
                base = PAD + c * NCH
                ps = pp.tile([96, NCH], F32, tag="cv", bufs=3)
                nc.tensor.matmul(ps[:], w0p_t[:],
                                 x0[:, base - 1:base - 1 + NCH],
                                 start=True, stop=False)
                yield
                nc.tensor.matmul(ps[:], w0c_t[:],
                                 x0[0:50, base + 1:base + 1 + NCH],
                                 start=False, stop=True)
                nc.scalar.activation(x1[:, base:base + NCH], ps[:],
                                     AF.Relu, bias=b0_t[:])
                yield
            x0_dma(s + 2)
            sample_state[s] = [x1, None, None, None]

        def gen_conv1(s, c):
            st = sample_state[s]
            x1 = st[0]
            if c == 0:
                x2 = xp.tile([128, SEG], BF16, tag="x2", bufs=4)
                nc.gpsimd.memset(x2[:, 0:PAD], 0.0)
                nc.gpsimd.memset(x2[:, PAD + 1000:SEG], 0.0)
                st[1] = x2
            x2 = st[1]
            base = PAD + c * NCH
            ps = pp.tile([128, NCH], F32, tag="cv", bufs=3)
            for t in range(5):
                nc.tensor.matmul(
                    ps[:], w1_t[:, t * 128:(t + 1) * 128],
                    x1[:, base + t - 2:base + t - 2 + NCH],
                    start=(t == 0), stop=(t == 4))
                if t < 4:
                    yield
            nc.scalar.activation(x2[:, base:base + NCH], ps[:],
                                 AF.Relu, bias=b1_t[:])
            yield

        def gen_conv2(s, c, half):
            st = sample_state[s]
            x2 = st[1]
            if half == 0:
                if c == 0:
                    st[2] = tmp.tile([128, 2], F32, tag="mxA", bufs=3,
                                     name="mxA")
                    st[3] = tmp.tile([72, 2], F32, tag="mxB", bufs=3,
                                     name="mxB")
                w_t, wid, mx = w2a_t, 128, st[2]
            else:
                w_t, wid, mx = w2b_t, 72, st[3]
            base = PAD + c * NCH
            ps = pp.tile([wid, NCH], F32, tag="cv", bufs=3)
            for t in range(7):
                nc.tensor.matmul(
                    ps[:], w_t[:, t * wid:(t + 1) * wid],
                    x2[:, base + t - 3:base + t - 3 + NCH],
                    start=(t == 0), stop=(t == 6))
                if t < 6:
                    yield
            nc.vector.reduce_max(mx[:, c:c + 1], ps[:],
                                 axis=mybir.AxisListType.X)
            if c == 1:
                dst = prT1p if half == 0 else prT2p
                nc.vector.reduce_max(dst[:, s:s + 1], mx[:],
                                     axis=mybir.AxisListType.X)
                if half == 1:
                    sample_state.pop(s)
            yield

        # FC head first-layer matmuls on the molecule embeddings can run
        # before the last conv pieces finish; woven in as mol pieces.
        # PSUM tiles allocated lazily (inside the generator) so the nt
        # slot rotation used by iter_pre/atom is not disturbed.
        fc_ps = {}

        def gen_fc_early():
            nc.gpsimd.tensor_copy(embT1[:], embT1f[:])
            nc.gpsimd.tensor_copy(embT2[:], embT2f[:])
            fc_ps["a"] = pp.tile([128, M], F32, tag="nt", bufs=2, name="ps0a")
            fc_ps["b"] = pp.tile([72, M], F32, tag="nt", bufs=2, name="ps0b")
            rhs2 = (embT1, embT2)
            for k in range(2):
                nc.tensor.matmul(fc_ps["a"][:], fc0_t[k][:, 0:128],
                                 rhs2[k][:], start=(k == 0), stop=False)
                yield
            for k in range(2):
                nc.tensor.matmul(fc_ps["b"][:], fc0_t[k][:, 128:200],
                                 rhs2[k][:], start=(k == 0), stop=False)
                yield

        # ---- build the two global piece streams and weave them ----
        conv_pieces = []
        mol_pieces = []
        for g in range(4):
            s0, s1, s2, s3 = (4 * g + i for i in range(4))
            mols = [GM * g + r for r in range(GM)]
            for stage in (gen_binput, gen_iter_pre, gen_iter_post,
                          gen_iter_pre, gen_iter_post, gen_atom):
                for m in mols:
                    mol_pieces.append(stage(m))
            conv_pieces += [
                gen_conv0(s0), gen_conv0(s1),
                gen_conv1(s0, 0), gen_conv1(s0, 1),
                gen_conv1(s1, 0), gen_conv1(s1, 1),
                gen_conv2(s0, 0, 0), gen_conv2(s0, 0, 1),
                gen_conv2(s0, 1, 0), gen_conv2(s0, 1, 1),
                gen_conv0(s2), gen_conv1(s2, 0), gen_conv1(s2, 1),
                gen_conv2(s1, 0, 0), gen_conv2(s1, 0, 1),
                gen_conv2(s1, 1, 0), gen_conv2(s1, 1, 1),
                gen_conv0(s3), gen_conv1(s3, 0), gen_conv1(s3, 1),
                gen_conv2(s2, 0, 0), gen_conv2(s2, 0, 1),
                gen_conv2(s2, 1, 0), gen_conv2(s2, 1, 1),
                gen_conv2(s3, 0, 0), gen_conv2(s3, 0, 1),
                gen_conv2(s3, 1, 0), gen_conv2(s3, 1, 1),
            ]
        mol_pieces.append(gen_fc_early())

        def stream(pieces):
            for p in pieces:
                yield from p

        cs = stream(conv_pieces)
        ms = stream(mol_pieces)
        # lead with the first mol group's binput+iter_pre (their inputs
        # land before any conv input does); warmup matmuls fill the
        # dependency stalls of this phase
        for _ in range(12):
            next(ms, None)
            emit_warm()
        conv_alive = mol_alive = True
        credit = 0.0
        RATIO = 672.0 / 276.0
        while conv_alive or mol_alive:
            credit += RATIO
            while credit >= 1.0 and conv_alive:
                credit -= 1.0
                if next(cs, StopIteration) is StopIteration:
                    conv_alive = False
            if mol_alive and next(ms, StopIteration) is StopIteration:
                mol_alive = False
            if not conv_alive:
                credit = 0.0

        # maxpool -> bias -> relu (monotone, so pool-first is exact)
        prT1 = sbs.tile([128, M], BF16, tag="prT1")
        nc.vector.tensor_scalar(prT1[:], prT1p[:], b2a_t[:], 0.0,
                                op0=ALU.add, op1=ALU.max)
        prT2 = sbs.tile([72, M], BF16, tag="prT2")
        nc.vector.tensor_scalar(prT2[:], prT2p[:], b2b_t[:], 0.0,
                                op0=ALU.add, op1=ALU.max)

        # ================= FC head (tail: protein contributions) =======
        rhs4 = (embT1, embT2, prT1, prT2)
        for k in (2, 3):
            nc.tensor.matmul(fc_ps["a"][:], fc0_t[k][:, 0:128], rhs4[k][:],
                             start=False, stop=(k == 3))
        h0a = tmp.tile([128, M], BF16, tag="h0a")
        nc.scalar.activation(h0a[:], fc_ps["a"][:], AF.Relu, bias=fc0ba_t[:])
        for k in (2, 3):
            nc.tensor.matmul(fc_ps["b"][:], fc0_t[k][:, 128:200], rhs4[k][:],
                             start=False, stop=(k == 3))
        h0b = tmp.tile([72, M], BF16, tag="h0b")
        nc.scalar.activation(h0b[:], fc_ps["b"][:], AF.Relu, bias=fc0bb_t[:])

        ps1 = pp.tile([100, M], F32, tag="nt", bufs=2)
        nc.tensor.matmul(ps1[:], fc1a_t[:], h0a[:], start=True, stop=False)
        nc.tensor.matmul(ps1[:], fc1b_t[:], h0b[:], start=False, stop=True)
        h1 = tmp.tile([100, M], BF16, tag="h1")
        nc.scalar.activation(h1[:], ps1[:], AF.Relu, bias=fc1bias_t[:])

        ps2 = pp.tile([1, M], F32, tag="nt", bufs=2)
        nc.tensor.matmul(ps2[:], fc2w_t[:], h1[:], start=True, stop=True)
        outsb = tmp.tile([1, M], F32, tag="outsb")
        nc.scalar.add(outsb[:], ps2[:], fc2b_t[:, 0:1])
        nc.sync.dma_start(d_out.ap(), outsb[:])

    nc.compile()
    return nc


def _prep(inputs):
    """Host preprocessing: returns the 8 per-core in_maps."""
    import ml_dtypes
    f32 = np.float32
    bf16 = ml_dtypes.bfloat16
    fatoms = np.asarray(inputs["fatoms"], f32)
    fbonds = np.asarray(inputs["fbonds"], f32)
    agraph = np.asarray(inputs["agraph"])
    bgraph = np.asarray(inputs["bgraph"])
    pseq = np.asarray(inputs["protein_seq"])
    W_i = np.asarray(inputs["W_i"], f32)
    W_h = np.asarray(inputs["W_h"], f32)
    W_o_w = np.asarray(inputs["W_o_w"], f32)
    W_o_b = np.asarray(inputs["W_o_b"], f32)
    embp = np.asarray(inputs["embed_protein"], f32)

    # protein embeddings, channel-major, with the shifted duplicate for
    # the packed conv0 (rows 50:100 = rows 0:50 shifted one col left)
    pvT = np.ascontiguousarray(embp[pseq].transpose(0, 2, 1))  # (B, 50, L)
    x0d = np.zeros((B, 100, SEG), bf16)
    x0d[:, 0:50, PAD:PAD + L] = pvT.astype(bf16)
    x0d[:, 50:100, PAD - 1:PAD - 1 + L] = x0d[:, 0:50, PAD:PAD + L]

    # adjacency one-hots (counts; contraction-dim-major)
    ar = np.arange(B)[:, None, None]
    cntB = np.zeros((B, NB, NB), f32)
    np.add.at(cntB, (ar, np.arange(NB)[None, :, None], bgraph), 1.0)
    abt = np.ascontiguousarray(cntB.transpose(0, 2, 1))        # (B, j, i)
    cntA = np.zeros((B, NA, NB), f32)
    np.add.at(cntA, (ar, np.arange(NA)[None, :, None], agraph), 1.0)
    aat = np.ascontiguousarray(cntA.transpose(0, 2, 1))        # (B, j, a)

    fbT = fbonds.transpose(0, 2, 1)                            # (B, 50, 96)
    faT = fatoms.transpose(0, 2, 1)                            # (B, 39, 48)
    cat1 = np.concatenate([faT, np.ones((B, 1, NA), f32)], axis=1)  # (B,40,48)

    # W_o scaled by 1/48: the atom mean becomes a plain sum (relu is
    # positively homogeneous)
    wo1 = np.zeros((40, 200), f32)
    wo1[:39] = W_o_w[0:39] / 48.0
    wo1[39] = W_o_b / 48.0
    wo2 = W_o_w[39:167] / 48.0
    wo3 = W_o_w[167:239] / 48.0

    conv_w = [np.asarray(inputs[f"conv{i}_w"], f32) for i in range(3)]
    conv_b = [np.asarray(inputs[f"conv{i}_b"], f32) for i in range(3)]
    w0p = np.concatenate([conv_w[0][:, :, 0].T, conv_w[0][:, :, 1].T], axis=0)
    w0c = np.ascontiguousarray(conv_w[0][:, :, 2].T)           # (50, 96)
    w1 = np.ascontiguousarray(conv_w[1].transpose(1, 2, 0))    # (96, 5, 128)
    w2 = np.ascontiguousarray(conv_w[2].transpose(1, 2, 0))    # (128, 7, 200)

    fcw = [np.asarray(inputs[f"fc{i}_w"], f32) for i in range(3)]
    fcb = [np.asarray(inputs[f"fc{i}_b"], f32) for i in range(3)]

    shared_bf = {
        "wi": W_i, "wha": W_h[0:128], "whb": W_h[128:200],
        "wo1": wo1, "wo2": wo2, "wo3": wo3,
        "w0p": w0p, "w0c": w0c, "w1": w1,
        "w2a": w2[:, :, 0:128], "w2b": w2[:, :, 128:200],
        "fc0a": fcw[0][0:128], "fc0b": fcw[0][128:200],
        "fc0c": fcw[0][200:328], "fc0d": fcw[0][328:400],
        "fc1a": fcw[1][0:128], "fc1b": fcw[1][128:200],
        "fc2w": fcw[2], "ones48": np.ones((48, 1), f32),
    }
    shared = {k: np.ascontiguousarray(v).astype(bf16)
              for k, v in shared_bf.items()}
    shared.update({
        "b0": conv_b[0].reshape(96, 1),
        "b1": conv_b[1].reshape(128, 1),
        "b2a": conv_b[2][0:128].reshape(128, 1),
        "b2b": conv_b[2][128:200].reshape(72, 1),
        "fc0ba": fcb[0][0:128].reshape(128, 1),
        "fc0bb": fcb[0][128:200].reshape(72, 1),
        "fc1bias": fcb[1].reshape(100, 1),
        "fc2b": fcb[2].reshape(1, 1),
    })
    for k in ("b0", "b1", "b2a", "b2b", "fc0ba", "fc0bb", "fc1bias", "fc2b"):
        shared[k] = np.ascontiguousarray(shared[k], f32)

    in_maps = []
    for c in range(NCORES):
        lo = c * M
        im = dict(shared)
        for g in range(M):
            im[f"x0d{g}"] = np.ascontiguousarray(x0d[lo + g])
        im["fbt"] = np.ascontiguousarray(
            fbT[lo:lo + M].transpose(1, 0, 2)).astype(bf16)
        im["cat1"] = np.ascontiguousarray(
            cat1[lo:lo + M].transpose(1, 0, 2)).astype(bf16)
        im["abt"] = np.ascontiguousarray(
            abt[lo:lo + M].transpose(1, 0, 2)).astype(bf16)
        im["aat"] = np.ascontiguousarray(
            aat[lo:lo + M].transpose(1, 0, 2)).astype(bf16)
        in_maps.append(im)
    return in_maps


def get_nc():
    if "nc" not in _CACHE:
        _CACHE["nc"] = _build_nc()
    return _CACHE["nc"]


def kernel(**inputs) -> np.ndarray:
    nc = get_nc()
    in_maps = _prep(inputs)
    res = run_bass_kernel_spmd(nc, in_maps, core_ids=list(range(NCORES)))
    outs = [res.results[c]["out"].reshape(M, 1) for c in range(NCORES)]
    return np.concatenate(outs, axis=0).astype(np.float32)
